# revision 26
# baseline (speedup 1.0000x reference)
"""CNNSummarizer (CNN encoder + 2-layer LSTM decoder + vocab projection) on 8 trn2 cores.

Sharding:
  - encoder: data-parallel over batch (4 batches per core); one AllGather of the
    per-batch encoder output enc (B, H) in bf16 (16KB payload).
  - LSTM recurrence: replicated on all 8 cores.
  - vocab projection (H -> V GEMM): column-sharded, 4000 vocab per core.

v2 design notes (vs the 1147us baseline):
  - everything bf16 on the PE (conv, fc, Xdec, recurrence, vocab): bf16 moving
    operands stream ~2 cols/cycle.
  - embedding tables passed as host-cast bf16, so gathers move half the bytes.
  - cell elementwise work split across DVE / Pool / ACT so no single engine
    serializes the chain; f*c products run on the (otherwise idle) Pool engine.
  - per-step gate-psum init is ONE bf16 identity-matmul of xs = xih[t] + xe
    (DVE pre-add) for layer 1, and of b2 for layer 2.
  - vocab bias is applied with a ones-row matmul accumulation (no obcast);
    evictions rotate DVE/Pool; logits are written bf16 and upcast on the host.
  - vocab units are paced into two PE insertion points per step so the PE never
    idles (keeps it in the 2.4GHz p-state).
"""

import math
from contextlib import ExitStack

import numpy as np

import concourse.bacc as bacc
import concourse.bass as bass
import concourse.mybir as mybir
import concourse.tile as tile
from concourse.masks import make_identity

V, E, H, F = 32000, 256, 512, 256
FS = (3, 4, 5)
B, S, T = 32, 512, 64
NCORES = 8
BL = B // NCORES          # batches per core
VS = V // NCORES          # vocab shard per core
TT = T - 1                # decode steps actually computed
G4 = 4 * H                # 2048 gates

dt = mybir.dt
F32 = dt.float32
BF16 = dt.bfloat16
AF = mybir.ActivationFunctionType
ALU = mybir.AluOpType
AX = mybir.AxisListType


def build(tt=TT, vs=VS, trace_sim=False):
    """Build the per-core program. All 8 cores run the same NEFF; sharding comes
    from per-core input values."""
    R = tt * B                       # rows of the (t, b) decode matrix
    NM = math.ceil(R / 128)          # m-chunks of decode rows
    NCH = NM                         # dec-token gather chunks (128 tokens each)
    RPAD = NM * 128
    NV = math.ceil(vs / 512)

    nc = bacc.Bacc("TRN2", target_bir_lowering=False, debug=False,
                   num_devices=NCORES)

    def inp(name, shape, dtype=BF16):
        return nc.dram_tensor(name, list(shape), dtype, kind="ExternalInput").ap()

    src_idx = inp("src_idx", (128, (BL * S) // 128), dt.int32)
    dec_idx = inp("dec_idx", (128, NCH), dt.int32)
    enc_emb = inp("enc_emb", (V, E))          # bf16 (host cast)
    dec_emb = inp("dec_emb", (V, E))          # bf16
    wconv = {k: inp(f"wconv{k}", (128, k * 4 * 128)) for k in FS}
    bconv = inp("bconv", (128, 2 * len(FS)), F32)   # col = fc*3 + k_idx
    fc1T = inp("fc1T", (128, 6 * H))
    fc1b = inp("fc1b", (1, H))
    fc2T = inp("fc2T", (128, 4 * H))
    fc2b = inp("fc2b", (1, H))
    WdT = inp("WdT", (128, 2 * G4))
    WeT = inp("WeT", (128, 4 * G4))
    b1row = inp("b1row", (1, G4))
    b2pack = inp("b2pack", (128, H))
    whh1T = inp("whh1T", (128, 4 * G4))
    wih2T = inp("wih2T", (128, 4 * G4))
    whh2T = inp("whh2T", (128, 4 * G4))
    owT = inp("owT", (128, 4 * vs))
    obrow = inp("obrow", (1, vs))

    out_dram = nc.dram_tensor("logits_sh", [R, vs], BF16,
                              kind="ExternalOutput").ap()

    with tile.TileContext(nc, trace_sim=trace_sim) as tc:
        with ExitStack() as ctx:
            dram = ctx.enter_context(tc.tile_pool(name="dram", bufs=1,
                                                  space="DRAM"))
            xih_dram = dram.tile([tt, 128, H], BF16)
            cc_in = dram.tile([BL, H], BF16)
            cc_out = dram.tile([B, H], BF16, addr_space="Shared")

            const = ctx.enter_context(tc.tile_pool(name="const", bufs=1))
            identF = const.tile([128, 128], F32)
            make_identity(nc, identF[:])
            ident_bf = const.tile([128, 128], BF16)
            nc.vector.tensor_copy(ident_bf[:], identF[:])
            scrF = const.tile([128, 128], F32)
            nc.vector.memset(scrF[:], 0.0)
            zpad = const.tile([128, 8], BF16)
            nc.vector.tensor_copy(zpad[:], scrF[:, 0:8])
            nc.vector.memset(scrF[0:1, :], 1.0)
            ones_bf = const.tile([1, 128], BF16)
            nc.vector.tensor_copy(ones_bf[:], scrF[0:1, :])
            ident32_bf = const.tile([32, 32], BF16)
            nc.vector.tensor_copy(ident32_bf[:], identF[0:32, 0:32])

            # index DMAs first: the gathers (and hence conv) gate phase 1
            rw = ctx.enter_context(tc.tile_pool(name="rw", bufs=1))
            idx_s_sb = rw.tile([128, (BL * S) // 128], dt.int32)
            nc.sync.dma_start(idx_s_sb[:], src_idx)
            idx_d_sb = rw.tile([128, NCH], dt.int32)
            nc.sync.dma_start(idx_d_sb[:], dec_idx)
            xe_packed = rw.tile([128, H], BF16)   # filled after AllGather

            # h2^T lives across recurrence + vocab phases
            big = ctx.enter_context(tc.tile_pool(name="big", bufs=1))
            h2T_all = big.tile([128, 4 * RPAD], BF16)   # [kc] blocks of h2^T

            # =========================================================
            # Phase 1: encoder (my BL batches) + Xdec GEMM (all rows)
            # =========================================================
            with ExitStack() as p1:
                wpool1 = p1.enter_context(tc.tile_pool(name="wpool1", bufs=1))
                gpool = p1.enter_context(tc.tile_pool(name="gpool", bufs=16))
                dgpool = p1.enter_context(tc.tile_pool(name="dgpool", bufs=16))
                tpp = p1.enter_context(tc.tile_pool(name="tpp", bufs=2,
                                                    space="PSUM"))
                cps = p1.enter_context(tc.tile_pool(name="cps", bufs=3,
                                                    space="PSUM"))
                fps = p1.enter_context(tc.tile_pool(name="fps", bufs=2,
                                                    space="PSUM"))
                xdpool = p1.enter_context(tc.tile_pool(name="xdpool", bufs=2))
                p1e = ExitStack()
                encp = p1e.enter_context(tc.tile_pool(name="encp", bufs=1))
                wconv_sb = {}
                for k in FS:
                    wk = encp.tile([128, k * 4 * 128], BF16,
                                   name=f"wconv{k}_sb")
                    nc.sync.dma_start(wk[:], wconv[k])
                    wconv_sb[k] = wk
                bconv_sb = encp.tile([128, 2 * len(FS)], F32)
                nc.sync.dma_start(bconv_sb[:], bconv)
                fc1T_sb = encp.tile([128, 6 * H], BF16)
                nc.sync.dma_start(fc1T_sb[:], fc1T)
                fc2T_sb = encp.tile([128, 4 * H], BF16)
                nc.sync.dma_start(fc2T_sb[:], fc2T)
                fc1b_sb = encp.tile([1, H], BF16)
                nc.sync.dma_start(fc1b_sb[:], fc1b)
                fc2b_sb = encp.tile([1, H], BF16)
                nc.sync.dma_start(fc2b_sb[:], fc2b)
                WdT_sb = wpool1.tile([128, 2 * G4], BF16)
                nc.sync.dma_start(WdT_sb[:], WdT)
                WeT_sb = wpool1.tile([128, 4 * G4], BF16)
                nc.sync.dma_start(WeT_sb[:], WeT)
                b1_sb = wpool1.tile([1, G4], BF16)
                nc.sync.dma_start(b1_sb[:], b1row)
                # big phase-2 weights stream behind the phase-1 weights
                whh1_sb = rw.tile([128, 4 * G4], BF16)
                nc.sync.dma_start(whh1_sb[:], whh1T)
                wih2_sb = rw.tile([128, 4 * G4], BF16)
                nc.sync.dma_start(wih2_sb[:], wih2T)
                whh2_sb = rw.tile([128, 4 * G4], BF16)
                nc.sync.dma_start(whh2_sb[:], whh2T)
                owT_sb = rw.tile([128, 4 * vs], BF16)
                nc.sync.dma_start(owT_sb[:], owT)
                obrow_sb = rw.tile([1, vs], BF16)
                nc.sync.dma_start(obrow_sb[:], obrow)
                b2_sb = rw.tile([128, H], BF16)
                nc.sync.dma_start(b2_sb[:], b2pack)

                XPAD = BL * (S + 8)
                xT_sb = encp.tile([128, 2 * XPAD], BF16)        # [ec] blocks
                dembT_sb = wpool1.tile([128, 2 * RPAD], BF16)   # [ec] blocks

                def evict(dst, src, parity):
                    if parity % 2 == 0:
                        nc.vector.tensor_copy(dst, src)
                    else:
                        nc.scalar.copy(dst, src)

                SEG = S + 8

                # ---- gathers interleave src/dec per batch on the Pool
                # queue, so conv b and Xdec chunks 4b..4b+3 both have data
                # by the time the PE reaches them ----
                gts = [None] * (4 * BL)
                dgts = [None] * NCH

                def src_gather(i):
                    gt = gpool.tile([128, E], BF16, tag="gath")
                    nc.gpsimd.indirect_dma_start(
                        out=gt[:], out_offset=None, in_=enc_emb,
                        in_offset=bass.IndirectOffsetOnAxis(
                            ap=idx_s_sb[:, i:i + 1], axis=0))
                    gts[i] = gt

                def dec_gather(m):
                    gt = dgpool.tile([128, E], BF16, tag="dgath")
                    nc.gpsimd.indirect_dma_start(
                        out=gt[:], out_offset=None, in_=dec_emb,
                        in_offset=bass.IndirectOffsetOnAxis(
                            ap=idx_d_sb[:, m:m + 1], axis=0))
                    dgts[m] = gt

                for i in range(4):
                    src_gather(i)
                for b in range(BL):
                    for m in range(4 * b, min(NCH, 4 * b + 4)):
                        dec_gather(m)
                    for i in range(4 * (b + 1), min(4 * BL, 4 * (b + 2))):
                        src_gather(i)

                pooled = encp.tile([128, 6 * BL], BF16)

                def conv_batch(b):
                    for ki, k in enumerate(FS):
                        for fc in range(2):
                            ps = cps.tile([128, 512], F32, tag="conv",
                                          space="PSUM")
                            first = True
                            for j in range(k):
                                for ec in range(2):
                                    lhs = wconv_sb[k][
                                        :, (j * 4 + ec * 2 + fc) * 128:
                                        (j * 4 + ec * 2 + fc) * 128 + 128]
                                    rhs = xT_sb[:, ec * XPAD + SEG * b + j:
                                                ec * XPAD + SEG * b + j + 512]
                                    nc.tensor.matmul(
                                        ps[:], lhs, rhs, start=first,
                                        stop=(j == k - 1 and ec == 1))
                                    first = False
                            kc = ki * 2 + fc
                            nc.vector.tensor_reduce(
                                pooled[:, BL * kc + b: BL * kc + b + 1],
                                ps[:, 0:S - k + 1], axis=AX.X, op=ALU.max)

                def xdec_chunk(m):
                    tm = min(4, tt - 4 * m)
                    Mm = 32 * tm
                    gt = dgts[m]
                    for ec in range(2):
                        tp = tpp.tile([128, 128], BF16, tag="tp",
                                      space="PSUM")
                        nc.tensor.transpose(
                            tp[:], gt[:, 128 * ec:128 * ec + 128],
                            ident_bf[:])
                        evict(dembT_sb[:, ec * RPAD + 128 * m:
                                       ec * RPAD + 128 * m + 128],
                              tp[:], m + ec)
                    xd_sb = xdpool.tile([128, G4], BF16, tag="xd_sb")
                    for n in range(4):
                        ps = fps.tile([128, 512], F32, tag="f", space="PSUM")
                        for ec in range(2):
                            nc.tensor.matmul(
                                ps[0:Mm, :],
                                dembT_sb[:, ec * RPAD + 128 * m:
                                         ec * RPAD + 128 * m + Mm],
                                WdT_sb[:, ec * G4 + 512 * n:
                                       ec * G4 + 512 * n + 512],
                                start=(ec == 0), stop=(ec == 1))
                        nc.vector.tensor_copy(
                            xd_sb[0:Mm, 512 * n:512 * n + 512], ps[0:Mm, :])
                    for tau in range(tm):
                        dst = xih_dram[4 * m + tau].rearrange(
                            "(j b) d -> b j d", j=4)
                        nc.sync.dma_start(dst,
                                          xd_sb[32 * tau:32 * tau + 32, :])

                for b in range(BL):
                    for ch in range(4):
                        gt = gts[4 * b + ch]
                        for ec in range(2):
                            tp = tpp.tile([128, 128], BF16, tag="tp",
                                          space="PSUM")
                            nc.tensor.transpose(
                                tp[:], gt[:, 128 * ec:128 * ec + 128],
                                ident_bf[:])
                            evict(xT_sb[:, ec * XPAD + SEG * b + 128 * ch:
                                        ec * XPAD + SEG * b + 128 * ch + 128],
                                  tp[:], ch + ec)
                    for ec in range(2):
                        nc.vector.tensor_copy(
                            xT_sb[:, ec * XPAD + SEG * b + S:
                                  ec * XPAD + SEG * (b + 1)], zpad[:])
                    conv_batch(b)

                for ki in range(len(FS)):
                    for fc in range(2):
                        kc = ki * 2 + fc
                        nc.scalar.activation(
                            pooled[:, BL * kc: BL * kc + BL],
                            pooled[:, BL * kc: BL * kc + BL],
                            AF.Relu, bias=bconv_sb[:, fc * 3 + ki: fc * 3 + ki + 1])

                # ---- fc1 -> relu -> fc2 -> AllGather(enc) ----
                ps1 = fps.tile([BL, H], F32, tag="f", space="PSUM")
                for kc in range(6):
                    nc.tensor.matmul(ps1[:], pooled[:, BL * kc: BL * kc + BL],
                                     fc1T_sb[:, H * kc: H * kc + H],
                                     start=(kc == 0), stop=False)
                nc.tensor.matmul(ps1[:], ones_bf[0:1, 0:BL], fc1b_sb[:],
                                 start=False, stop=True)
                h1e = encp.tile([BL, H], BF16)
                nc.scalar.activation(h1e[:], ps1[:], AF.Relu)

                h1eT = encp.tile([128, 4 * BL], BF16)
                for kc in range(4):
                    tp = tpp.tile([128, 128], BF16, tag="tp", space="PSUM")
                    nc.tensor.transpose(tp[0:128, 0:BL],
                                        h1e[:, 128 * kc:128 * kc + 128],
                                        ident_bf[0:BL, 0:BL])
                    nc.vector.tensor_copy(h1eT[:, BL * kc:BL * kc + BL],
                                          tp[0:128, 0:BL])

                ps2 = fps.tile([BL, H], F32, tag="f", space="PSUM")
                for kc in range(4):
                    nc.tensor.matmul(ps2[:], h1eT[:, BL * kc:BL * kc + BL],
                                     fc2T_sb[:, H * kc:H * kc + H],
                                     start=(kc == 0), stop=False)
                nc.tensor.matmul(ps2[:], ones_bf[0:1, 0:BL], fc2b_sb[:],
                                 start=False, stop=True)
                enc_sb = encp.tile([BL, H], BF16)
                nc.vector.tensor_copy(enc_sb[:], ps2[:])
                nc.sync.dma_start(cc_in[:], enc_sb[:])
                p1e.close()

                nc.gpsimd.collective_compute(
                    "AllGather", ALU.bypass,
                    replica_groups=[list(range(NCORES))],
                    ins=[cc_in.opt()], outs=[cc_out.opt()])

                # ---- Xdec GEMM -> packed xih_dram (independent of the
                # AllGather, runs on the PE while the mesh is in flight) ----
                for m in range(NM):
                    xdec_chunk(m)

                # ---- xe = enc_all @ WeT + b1, built directly in the packed
                # (32*gateblock+b, d) layout (stalls on the AllGather) ----
                enc_all = wpool1.tile([B, H], BF16)
                nc.sync.dma_start(enc_all[:], cc_out)
                encT = wpool1.tile([128, 4 * B], BF16)
                for kc in range(4):
                    tp = tpp.tile([128, 128], BF16, tag="tp", space="PSUM")
                    nc.tensor.transpose(tp[0:128, 0:B],
                                        enc_all[:, 128 * kc:128 * kc + 128],
                                        ident_bf[0:B, 0:B])
                    nc.vector.tensor_copy(encT[:, B * kc:B * kc + B],
                                          tp[0:128, 0:B])
                for gb in range(4):
                    ps = fps.tile([B, 512], F32, tag="f", space="PSUM")
                    for kc in range(4):
                        nc.tensor.matmul(
                            ps[:], encT[:, B * kc:B * kc + B],
                            WeT_sb[:, kc * G4 + 512 * gb:
                                   kc * G4 + 512 * gb + 512],
                            start=(kc == 0), stop=False)
                    nc.tensor.matmul(ps[:], ones_bf[0:1, 0:B],
                                     b1_sb[:, 512 * gb:512 * gb + 512],
                                     start=False, stop=True)
                    nc.vector.tensor_copy(
                        xe_packed[32 * gb:32 * gb + 32, :], ps[:])

            # =========================================================
            # Phase 2: recurrence with packed gate PSUM, col-tiled GEMMs
            # gate-block order [i, f, o, g] on psum partitions [0:32,...]
            # g-block weights are host-doubled: tanh(g) = 2*sig(2g) - 1,
            # so ONE 128-partition sigmoid covers all gates.
            # =========================================================
            with ExitStack() as p2:
                rp = p2.enter_context(tc.tile_pool(name="rp", bufs=2))
                xp = p2.enter_context(tc.tile_pool(name="xp", bufs=3))
                rps = p2.enter_context(tc.tile_pool(name="rps", bufs=2,
                                                    space="PSUM"))
                tps = p2.enter_context(tc.tile_pool(name="tps", bufs=1,
                                                    space="PSUM"))
                vo = p2.enter_context(tc.tile_pool(name="vo", bufs=3))
                vps = p2.enter_context(tc.tile_pool(name="vps", bufs=3,
                                                    space="PSUM"))

                c1 = rp.tile([32, H], BF16, tag="c1")
                nc.vector.memset(c1[:], 0.0)
                c2 = rp.tile([32, H], BF16, tag="c2")
                nc.vector.memset(c2[:], 0.0)

                def cell(ps_g, c_prev, tag):
                    """LSTM cell from packed-gate psum (128, H) -> (h, c_new).

                    Gate order [f, i, o, g]. HW rule: 2-input DVE/Pool ops need
                    both inputs at the SAME start partition; ACT may cross. c
                    lives at base 0 (f's home); tanh(g) re-homes to [32:64]
                    (i's home) via ACT, th to [64:96] (o's home). Every
                    DVE/Pool output writes partition base 0 (the fast path).
                    f*c runs on the idle Pool engine.
                    """
                    sig = rp.tile([96, H], BF16, tag=f"sig{tag}")
                    nc.scalar.activation(sig[:], ps_g[0:96, :], AF.Sigmoid)
                    tg = rp.tile([64, H], BF16, tag=f"tg{tag}")
                    nc.scalar.activation(tg[32:64, :], ps_g[96:128, :], AF.Tanh)
                    m2 = rp.tile([32, H], BF16, tag=f"m2{tag}")
                    nc.gpsimd.tensor_mul(m2[:], sig[0:32, :], c_prev[:])
                    m1 = rp.tile([32, H], BF16, tag=f"m1{tag}")
                    nc.vector.tensor_mul(m1[:], tg[32:64, :], sig[32:64, :])
                    c_new = rp.tile([32, H], BF16, tag=f"c{tag}")
                    nc.vector.tensor_add(c_new[:], m1[:], m2[:])
                    th = rp.tile([96, H], BF16, tag=f"th{tag}")
                    nc.scalar.activation(th[64:96, :], c_new[:], AF.Tanh)
                    h = rp.tile([32, H], BF16, tag=f"h{tag}")
                    nc.vector.tensor_mul(h[:], sig[64:96, :], th[64:96, :])
                    return h, c_new

                def transpose_state(h, dsts):
                    """h (32, H) -> one (128, 128) psum of h^T chunks, then a
                    single strided evict per destination."""
                    tp = tps.tile([128, 128], BF16, tag="tps",
                                  space="PSUM")
                    for kc in range(4):
                        nc.tensor.transpose(tp[:, 32 * kc:32 * kc + 32],
                                            h[:, 128 * kc:128 * kc + 128],
                                            ident32_bf[:])
                    for dst in dsts:
                        nc.vector.tensor_copy(dst, tp[:])
                    return tp

                def gemm_block(ps, stat, stat_base, w_sb, final):
                    for kc in range(4):
                        for j in range(4):
                            nc.tensor.matmul(
                                ps[32 * j:32 * j + 32, :],
                                stat[:, stat_base(kc): stat_base(kc) + 32],
                                w_sb[:, kc * G4 + 512 * j:
                                     kc * G4 + 512 * j + 512],
                                start=False,
                                stop=(final and kc == 3 and j == 3),
                                skip_group_check=True,
                                tile_position=(0, 32 * j))

                def vocab_unit(m, n, parity):
                    """One (row-chunk, vocab-tile) unit of the output GEMM.
                    Bias rides a ones-row matmul; evict engine by parity."""
                    Mm = min(128, R - 128 * m)
                    nw = min(512, vs - 512 * n)
                    ps = vps.tile([128, 512], F32, tag="vps", space="PSUM")
                    for kc in range(4):
                        nc.tensor.matmul(
                            ps[0:Mm, 0:nw],
                            h2T_all[:, kc * RPAD + 128 * m:
                                    kc * RPAD + 128 * m + Mm],
                            owT_sb[:, kc * vs + 512 * n: kc * vs + 512 * n + nw],
                            start=(kc == 0), stop=False)
                    nc.tensor.matmul(
                        ps[0:Mm, 0:nw], ones_bf[0:1, 0:Mm],
                        obrow_sb[:, 512 * n:512 * n + nw],
                        start=False, stop=True)
                    ob = vo.tile([128, 512], BF16, tag="ob")
                    if parity % 2 == 0:
                        nc.vector.tensor_copy(ob[0:Mm, 0:nw], ps[0:Mm, 0:nw])
                    else:
                        nc.scalar.copy(ob[0:Mm, 0:nw], ps[0:Mm, 0:nw])
                    nc.sync.dma_start(
                        out_dram[128 * m:128 * m + Mm, 512 * n:512 * n + nw],
                        ob[0:Mm, 0:nw])

                vunits = [(m, n) for m in range(NM) for n in range(NV)]
                vemitted = 0

                h2T_view = h2T_all[:].rearrange("p (c r) -> p c r", c=4)

                def drain_vocab(limit, avail, parity):
                    nonlocal vemitted
                    while (vemitted < limit and vemitted < len(vunits)
                           and vunits[vemitted][0] < avail):
                        vocab_unit(*vunits[vemitted], parity)
                        vemitted += 1

                # ---- prologue: step 0 cell1 (no h contribution) ----
                xih_t = xp.tile([128, H], BF16, tag="xih")
                nc.sync.dma_start(xih_t[:], xih_dram[0])
                xs = xp.tile([128, H], BF16, tag="xs")
                nc.vector.tensor_add(xs[:], xih_t[:], xe_packed[:])
                ps_g1 = rps.tile([128, H], F32, tag="g1", space="PSUM")
                nc.tensor.matmul(ps_g1[:], ident_bf[:], xs[:],
                                 start=True, stop=True, skip_group_check=True)
                h1, c1 = cell(ps_g1, c1, "1")
                xih_t = xp.tile([128, H], BF16, tag="xih")
                nc.sync.dma_start(xih_t[:], xih_dram[1])

                def h2T_store(h2_prev, t_prev):
                    tpv = tps.tile([128, 128], BF16, tag="tps",
                                   space="PSUM")
                    for kc in range(4):
                        nc.tensor.transpose(tpv[:, 32 * kc:32 * kc + 32],
                                            h2_prev[:, 128 * kc:128 * kc + 128],
                                            ident32_bf[:])
                    nc.vector.tensor_copy(
                        h2T_view[:, :, 32 * t_prev:32 * t_prev + 32],
                        tpv[:].rearrange("p (c r) -> p c r", c=4))

                # Software pipeline: iter t emits in PE order
                #   h1T_t transpose        (cell1_t finished long ago)
                #   ps_g1(t+1) gemm        -> cell1_{t+1} chain
                #   vocab unit             (filler)
                #   h2T_{t-1} transpose    (cell2_{t-1} had a full iter of slack)
                #   ps_g2(t) gemm          -> cell2_t chain
                #   vocab units            (filler while cell2_t runs)
                # No PE op ever directly trails the cell it depends on, so the
                # PE stays continuously busy (keeps the 2.4GHz p-state).
                h2_prev = None
                for t in range(tt):
                    avail = min(NM, t // 4)
                    quota = min(len(vunits), max(0, int(2.4 * (t - 3))),
                                vemitted + 2)

                    h1T = rp.tile([128, 128], BF16, tag="h1T")
                    transpose_state(h1, [h1T[:]])

                    if t + 1 < tt:
                        # xs for t+1; xih_{t+1} was prefetched last iter
                        xs = xp.tile([128, H], BF16, tag="xs")
                        nc.gpsimd.tensor_add(xs[:], xih_t[:], xe_packed[:])
                        ps_g1 = rps.tile([128, H], F32, tag="g1", space="PSUM")
                        nc.tensor.matmul(ps_g1[:], ident_bf[:], xs[:],
                                         start=True, stop=False,
                                         skip_group_check=True)
                        gemm_block(ps_g1, h1T, lambda kc: 32 * kc, whh1_sb,
                                   True)

                    if h2_prev is not None:
                        h2T_store(h2_prev, t - 1)

                    ps_g2 = rps.tile([128, H], F32, tag="g2", space="PSUM")
                    nc.tensor.matmul(ps_g2[:], ident_bf[:], b2_sb[:],
                                     start=True, stop=False,
                                     skip_group_check=True)
                    if t > 0:
                        gemm_block(ps_g2, h2T_all,
                                   lambda kc, _t=t: kc * RPAD + 32 * (_t - 1),
                                   whh2_sb, False)
                    gemm_block(ps_g2, h1T, lambda kc: 32 * kc, wih2_sb, True)

                    if t + 1 < tt:
                        # cell1_{t+1} chain starts as soon as ps_g1 is done
                        h1, c1 = cell(ps_g1, c1, "1")

                    if t + 2 < tt:
                        xih_t = xp.tile([128, H], BF16, tag="xih")
                        nc.sync.dma_start(xih_t[:], xih_dram[t + 2])

                    # cell2_t chain on ACT/DVE/Pool while PE runs vocab
                    h2, c2 = cell(ps_g2, c2, "2")
                    h2_prev = h2

                    drain_vocab(quota, avail, t)

                h2T_store(h2_prev, tt - 1)
                while vemitted < len(vunits):
                    vocab_unit(*vunits[vemitted], vemitted)
                    vemitted += 1

    nc.compile()
    return nc


# =====================================================================
# Host side
# =====================================================================

def _bf16(a):
    import ml_dtypes
    return np.ascontiguousarray(np.asarray(a, dtype=np.float32).astype(ml_dtypes.bfloat16))


def _chunk(a):
    """(c*128, X) -> (128, c*X): partition-chunked layout for SBUF tiles."""
    c = a.shape[0] // 128
    return np.ascontiguousarray(
        a.reshape(c, 128, -1).transpose(1, 0, 2).reshape(128, -1))


def host_prep(inputs, tt=TT, vs=VS):
    """Build per-core input maps from the full problem inputs."""
    R = tt * B
    NM = math.ceil(R / 128)
    f32 = lambda a: np.ascontiguousarray(np.asarray(a), dtype=np.float32)
    # gate permutation [f, i, o, g] (f first so c pairs with it at base 0)
    perm = np.concatenate([np.arange(H, 2 * H), np.arange(0, H),
                           np.arange(3 * H, 4 * H), np.arange(2 * H, 3 * H)])

    src = np.asarray(inputs["src"])
    trg = np.asarray(inputs["trg"])

    w_ih1 = f32(inputs["w_ih1"])[perm]
    b1 = (f32(inputs["b_ih1"]) + f32(inputs["b_hh1"]))[perm][None, :]
    b2 = (f32(inputs["b_ih2"]) + f32(inputs["b_hh2"]))[perm]
    b2pack = np.ascontiguousarray(
        np.broadcast_to(b2.reshape(4, 1, H), (4, 32, H)).reshape(128, H))

    shared = {
        "enc_emb": _bf16(inputs["enc_emb"]),
        "dec_emb": _bf16(inputs["dec_emb"]),
        "bconv": np.ascontiguousarray(
            np.stack([f32(inputs[f"conv_b{k}"]).reshape(2, 128)[fc]
                      for fc in range(2) for k in FS], axis=1)),
        "fc1T": _bf16(_chunk(f32(inputs["fc1_w"]).T)),
        "fc1b": _bf16(f32(inputs["fc1_b"])[None, :]),
        "fc2T": _bf16(_chunk(f32(inputs["fc2_w"]).T)),
        "fc2b": _bf16(f32(inputs["fc2_b"])[None, :]),
        "WdT": _bf16(_chunk(np.ascontiguousarray(w_ih1[:, :E].T))),
        "WeT": _bf16(_chunk(np.ascontiguousarray(w_ih1[:, E:].T))),
        "b1row": _bf16(b1), "b2pack": _bf16(b2pack),
        "whh1T": _bf16(_chunk(np.ascontiguousarray(f32(inputs["w_hh1"])[perm].T))),
        "wih2T": _bf16(_chunk(np.ascontiguousarray(f32(inputs["w_ih2"])[perm].T))),
        "whh2T": _bf16(_chunk(np.ascontiguousarray(f32(inputs["w_hh2"])[perm].T))),
    }
    for k in FS:
        A = f32(inputs[f"conv_w{k}"]).transpose(2, 1, 0)   # (k, E, F)
        A = A.reshape(k, 2, 128, 2, 128).transpose(0, 1, 3, 2, 4)
        shared[f"wconv{k}"] = _bf16(_chunk(A.reshape(k * 4 * 128, 128)))

    dtoks = trg[:, :tt].T.reshape(-1).astype(np.int32)
    dtoks = np.concatenate([dtoks, np.zeros(NM * 128 - R, np.int32)])
    dec_idx = np.ascontiguousarray(dtoks.reshape(NM, 128).T)

    owT_full = np.ascontiguousarray(f32(inputs["out_w"]).T)   # (H, V)
    ob_full = f32(inputs["out_b"])

    in_maps = []
    for c in range(NCORES):
        stoks = src[BL * c: BL * (c + 1)].reshape(-1).astype(np.int32)
        m = dict(shared)
        m["src_idx"] = np.ascontiguousarray(stoks.reshape(-1, 128).T)
        m["dec_idx"] = dec_idx
        m["owT"] = _bf16(_chunk(np.ascontiguousarray(
            owT_full[:, vs * c: vs * (c + 1)])))
        m["obrow"] = _bf16(ob_full[None, vs * c: vs * (c + 1)])
        in_maps.append(m)
    return in_maps


def assemble(results, tt=TT, vs=VS):
    """Gather per-core logit shards -> full (B, T, V) output."""
    import ml_dtypes
    out = np.zeros((B, T, V), dtype=np.float32)
    for c, res in enumerate(results):
        sh = np.asarray(res["logits_sh"])
        if sh.dtype == np.uint16 or sh.dtype.itemsize == 2:
            sh = sh.view(ml_dtypes.bfloat16)
        sh = sh.astype(np.float32).reshape(tt, B, vs)
        out[:, 1:1 + tt, vs * c: vs * (c + 1)] = sh.transpose(1, 0, 2)
    return out


_CACHE = {}


def kernel(**inputs):
    if "nc" not in _CACHE:
        _CACHE["nc"] = build()
    nc = _CACHE["nc"]
    from concourse.bass_utils import run_bass_kernel_spmd
    in_maps = host_prep(inputs)
    res = run_bass_kernel_spmd(nc, in_maps, core_ids=list(range(NCORES)))
    return assemble(res.results)
